# revision 17
# baseline (speedup 1.0000x reference)
"""Causal multi-head attention on 8 trn2 NeuronCores.

Problem: B=2, S=2048, D=2048, H=16 (HD=128), fp32 in/out.
Sharding: tensor-parallel over heads - core c owns heads {2c, 2c+1} for both
batches. Each core computes its Q/K/V projections, attention for its 4
(batch, head) pairs, and a partial output projection over its head slice.
The host sums the 8 partial outputs and adds the output bias.

v3 design (v1 ~470us, v2 ~368us):
  - fp16 operands everywhere (PE runs fp16 at 1 cycle/row like f32r, but DMA
    traffic halves and DVE gets 2x/4x modes). PSUM accumulation stays fp32.
  - Q^T/K^T/V all SBUF-resident in fp16 (no DRAM spill).
  - Fine-grained causal diagonal: diagonal 512x512 score blocks are computed
    with per-128-chunk shrinking free dims (512/384/256/128), for scores,
    exp and PV alike.
  - Softmax denominator off the PE: E chunks are pair-folded on DVE (wide
    fp16 adds) as they are produced, a small post-loop tree finishes the
    fold, and a single ones-matmul per (b,h,qb) does the partition-sum
    (v2 spent ~29us of PE on per-chunk ones-matmuls).
  - The extended causal mask (zero below-range + triangle) runs on GPSIMD
    and also zeroes the never-written columns of diagonal E chunks so the
    fold may read full 512-wide rows.
  - Software-pipelined phase B: per k-chunk t we issue score matmuls + exp,
    the PV matmuls of chunk t-1, a pair-fold at odd t, and 1-4 pending
    output-projection tiles from the PREVIOUS query block, keeping the PE
    dense while ACT does the exp chain and DVE/ACT do PSUM evacuation.
  - Per-engine balancing: phase-A PSUM->SBUF copies on ACT (idle in A);
    phase-C copies split 1/3 ACT, 2/3 DVE; reciprocal is the single-op DVE
    approx (~18 bits); masks+fold on GPSIMD/DVE.
"""

import os

import numpy as np

import concourse.bacc as bacc
import concourse.tile as tile
from concourse import mybir
from concourse.bass_utils import run_bass_kernel_spmd


def _install_neff_cache():
    """Cache compiled NEFFs on disk keyed by BIR content hash.

    Purely a compile-time memo: identical BIR -> identical NEFF, so repeat
    runs skip the multi-minute neuronxcc compile. No effect on execution.
    """
    import hashlib
    import shutil

    import concourse.bass2jax as _b2j
    import concourse.bass_utils as _bu

    if getattr(_bu, "_neff_cache_installed", False):
        return
    cache_dir = os.environ.get("NEFF_CACHE_DIR", "/tmp/neff_cache")
    orig = _bu.compile_bir_kernel

    def cached(bir_json, tmpdir, neff_name="file.neff"):
        try:
            os.makedirs(cache_dir, exist_ok=True)
            key = hashlib.sha256(bir_json).hexdigest()[:24]
            cpath = os.path.join(cache_dir, key + ".neff")
            dst = os.path.join(tmpdir, neff_name)
            if os.path.exists(cpath):
                shutil.copy(cpath, dst)
                return dst
            out = orig(bir_json, tmpdir, neff_name)
            shutil.copy(out, cpath)
            return out
        except OSError:
            return orig(bir_json, tmpdir, neff_name)

    _bu.compile_bir_kernel = cached
    _b2j.compile_bir_kernel = cached
    _bu._neff_cache_installed = True


_install_neff_cache()

B, S, D, H = 2, 2048, 2048, 16
HD = D // H          # 128
NCORES = 8
HPC = H // NCORES    # heads per core = 2
T = B * S            # 4096 total token rows
KO = D // 128        # 16 contraction chunks
NTB = T // 512       # 8 phase-A token blocks of 512
NQB = S // 512       # 4 query blocks per batch
SCALE = 1.0 / float(np.sqrt(HD))

_built = {}


def _build(with_bias):
    f32 = mybir.dt.float32
    f16 = mybir.dt.float16
    Exp = mybir.ActivationFunctionType.Exp
    Copy = mybir.ActivationFunctionType.Copy

    nc = bacc.Bacc(None, target_bir_lowering=False)

    # ---- per-core DRAM parameters (host supplies per-core shards) ----
    xt_p = nc.declare_dram_parameter("XT", [KO, 128, T], f16, False)
    wqt_p = nc.declare_dram_parameter("WQT", [KO, 128, HPC * HD], f16, False)
    wkt_p = nc.declare_dram_parameter("WKT", [KO, 128, HPC * HD], f16, False)
    wvt_p = nc.declare_dram_parameter("WVT", [KO, 128, HPC * HD], f16, False)
    wot_p = nc.declare_dram_parameter("WOT", [128, HPC, D], f16, False)
    bias_p = nc.declare_dram_parameter("BIAS", [1, 3, HPC * HD], f16, False)
    trix_p = nc.declare_dram_parameter("TRIX", [128, 4, 512], f16, False)
    ones_p = nc.declare_dram_parameter("ONES", [128, 512], f16, False)
    out_p = nc.declare_dram_parameter("OUT", [B, S, D], f16, True)

    with tile.TileContext(nc) as tc:
        with tc.tile_pool(name="persist", bufs=1) as persist:
            # SBUF-resident projections: Q^T/K^T [hd, tokens], V [tokens, hd]
            qt_res = persist.tile([128, B, HPC, S], f16)
            kt_res = persist.tile([128, B, HPC, S], f16)
            v_res = persist.tile([128, B, HPC, S // 128, HD], f16)
            ones_t = persist.tile([128, 512], f16)

            # ---------------- Phase A: projections ----------------
            with (
                tc.tile_pool(name="wqkv", bufs=1) as wpool,
                tc.tile_pool(name="xs", bufs=4) as xpool,
                tc.tile_pool(name="psA", bufs=4, space="PSUM") as psA,
            ):
                wq = wpool.tile([128, KO, HPC * HD], f16, tag="wq")
                wk = wpool.tile([128, KO, HPC * HD], f16, tag="wk")
                wv = wpool.tile([128, KO, HPC * HD], f16, tag="wv")
                nc.sync.dma_start(ones_t, ones_p[:])
                for g in range(8):
                    ksl = slice(g * 2, (g + 1) * 2)
                    nc.sync.dma_start(
                        wq[:, ksl], wqt_p[ksl].rearrange("ko p m -> p ko m")
                    )
                if with_bias:
                    bias = wpool.tile([1, 3, HPC * HD], f16, tag="bias")
                    nc.sync.dma_start(bias, bias_p[:])
                ones_row = ones_t[0:1, :]

                # PE warm-up: ~7us of throwaway matmuls while the first X/W
                # blocks stream in, so the HAM clock gate reaches 8/8 before
                # the real matmuls start (saves the 1.2GHz cold ramp)
                ps_warm = psA.tile([128, 512], f32, tag="qk")
                for _ in range(32):
                    nc.tensor.matmul(
                        ps_warm, lhsT=ones_t[:, :128], rhs=ones_t,
                        start=True, stop=True,
                    )

                for tb in range(NTB):
                    b = (tb * 512) // S
                    s0 = (tb * 512) % S
                    xt_h = []
                    for half in range(2):
                        xth = xpool.tile([128, KO // 2, 512], f16, tag="xt")
                        if tb == 0:
                            # fine-grained first loads: the first matmul only
                            # needs wq-g0 + the first ko chunk, so let it
                            # start as soon as those land
                            for g in range(8):
                                k0 = half * 8 + g
                                nc.sync.dma_start(
                                    xth[:, g : g + 1],
                                    xt_p[k0 : k0 + 1, :, 0:512].rearrange(
                                        "ko p t -> p ko t"
                                    ),
                                )
                        else:
                            for g in range(2):
                                k0 = half * 8 + g * 4
                                nc.sync.dma_start(
                                    xth[:, g * 4 : (g + 1) * 4],
                                    xt_p[
                                        k0 : k0 + 4, :, tb * 512 : (tb + 1) * 512
                                    ].rearrange("ko p t -> p ko t"),
                                )
                        xt_h.append(xth)

                    if tb == 0:
                        # wk/wv queued after tb0's X block: they land during
                        # tb0's Q matmuls instead of delaying the first one
                        for g in range(4):
                            ksl = slice(g * 4, (g + 1) * 4)
                            nc.sync.dma_start(
                                wk[:, ksl], wkt_p[ksl].rearrange("ko p m -> p ko m")
                            )
                            nc.sync.dma_start(
                                wv[:, ksl], wvt_p[ksl].rearrange("ko p m -> p ko m")
                            )

                    def xt_at(ko):
                        return xt_h[ko // 8][:, ko % 8]

                    # Q^T and K^T: [hd, tokens] per head
                    for (wt, dst, bi) in ((wq, qt_res, 0), (wk, kt_res, 1)):
                        for h in range(HPC):
                            ps = psA.tile([128, 512], f32, tag="qk")
                            for ko in range(KO):
                                nc.tensor.matmul(
                                    ps,
                                    lhsT=wt[:, ko, h * HD : (h + 1) * HD],
                                    rhs=xt_at(ko),
                                    start=(ko == 0),
                                    stop=(ko == KO - 1) and not with_bias,
                                )
                            if with_bias:
                                nc.tensor.matmul(
                                    ps,
                                    lhsT=bias[:, bi, h * HD : (h + 1) * HD],
                                    rhs=ones_row,
                                    start=False,
                                    stop=True,
                                )
                            nc.scalar.activation(
                                dst[:, b, h, s0 : s0 + 512], ps, Copy
                            )
                    # V: [tokens, hd] natural layout
                    for tsub in range(4):
                        ps = psA.tile([128, HPC * HD], f32, tag="v")
                        for ko in range(KO):
                            nc.tensor.matmul(
                                ps,
                                lhsT=xt_at(ko)[:, tsub * 128 : (tsub + 1) * 128],
                                rhs=wv[:, ko],
                                start=(ko == 0),
                                stop=(ko == KO - 1) and not with_bias,
                            )
                        if with_bias:
                            nc.tensor.matmul(
                                ps,
                                lhsT=ones_row[:, :128],
                                rhs=bias[:, 2],
                                start=False,
                                stop=True,
                            )
                        sc = (s0 + tsub * 128) // 128
                        nc.scalar.activation(
                            v_res[:, b, :, sc, :],
                            ps.rearrange("p (h d) -> p h d", h=HPC),
                            Copy,
                        )

            # ------------- Phase B + C: attention + out projection -------------
            # PSUM pool open order controls bank placement: psO lands on banks
            # phase A's psA just used; psO's first use is latest in phase B,
            # giving psA's tail time to drain without stalling.
            with (
                tc.tile_pool(name="psO", bufs=2, space="PSUM") as psO,
                tc.tile_pool(name="psD", bufs=2, space="PSUM") as psD,
                tc.tile_pool(name="psC", bufs=2, space="PSUM") as psC,
                tc.tile_pool(name="psS", bufs=2, space="PSUM") as psS,
                tc.tile_pool(name="bconst", bufs=1) as bconst,
                tc.tile_pool(name="epool", bufs=4) as epool,
                tc.tile_pool(name="accp", bufs=2) as accp,
                tc.tile_pool(name="ctx", bufs=3) as ctxp,
                tc.tile_pool(name="recp", bufs=2) as recp,
                tc.tile_pool(name="obp", bufs=6) as obp,
            ):
                trix = bconst.tile([128, 4, 512], f16, tag="trix")
                nc.sync.dma_start(trix, trix_p[:])
                wot = bconst.tile([128, HPC, D], f16, tag="wot")
                nc.sync.dma_start(wot, wot_p[:])
                ones_cm = ones_t[:, :128]

                copy_idx = [0]
                # pending output-projection tiles (closures), drained into the
                # next query block's t-loop to keep the PE dense
                pending = []

                def push_phC(b, qbo, ctxs):
                    for qc in range(4 * qbo, 4 * qbo + 4):
                        for oc in range(D // 512):
                            def emit(b=b, qc=qc, oc=oc, ctxs=ctxs):
                                pso = psO.tile([128, 512], f32, tag="o")
                                for h in range(HPC):
                                    nc.tensor.matmul(
                                        pso,
                                        lhsT=ctxs[h][:, qc * 128 : (qc + 1) * 128],
                                        rhs=wot[:, h, oc * 512 : (oc + 1) * 512],
                                        start=(h == 0),
                                        stop=(h == HPC - 1),
                                    )
                                ob = obp.tile([128, 512], f16, tag="ob")
                                if copy_idx[0] % 3 == 0:
                                    nc.scalar.activation(ob, pso, Copy)
                                else:
                                    nc.vector.tensor_copy(ob, pso)
                                copy_idx[0] += 1
                                nc.sync.dma_start(
                                    out_p[
                                        b,
                                        qc * 128 : (qc + 1) * 128,
                                        oc * 512 : (oc + 1) * 512,
                                    ],
                                    ob,
                                )
                            pending.append(emit)

                def drain_phC(n):
                    while n > 0 and pending:
                        pending.pop(0)()
                        n -= 1

                for b in range(B):
                    ctxs = [
                        ctxp.tile([128, S], f16, tag="ctxT", name=f"ctx{b}_{h}")
                        for h in range(HPC)
                    ]

                    for qb in range(NQB):
                        nk = 4 * (qb + 1)
                        emt = [
                            epool.tile(
                                [128, KO, 512], f16, tag="e", name=f"e{b}_{qb}_{h}"
                            )
                            for h in range(HPC)
                        ]
                        # fold accumulator [h, chunk, q]
                        acc = accp.tile(
                            [128, HPC, KO // 2, 512], f16, tag="acc",
                            name=f"acc{b}_{qb}",
                        )
                        pscs = [
                            psC.tile([128, 512], f32, tag="c", name=f"c{b}_{qb}_{h}")
                            for h in range(HPC)
                        ]
                        # zero the never-written low columns of diagonal E
                        # chunks so the denominator fold can read full rows
                        for h in range(HPC):
                            for i in range(1, 4):
                                nc.gpsimd.memset(
                                    emt[h][:, 4 * qb + i, : 128 * i], 0.0
                                )

                        def off_of(t, qb=qb):
                            i = t - 4 * qb
                            return 128 * i if i > 0 else 0

                        def issue_pv(t, qb=qb, nk=nk, emt=emt, pscs=pscs, b=b):
                            off = off_of(t, qb)
                            for h in range(HPC):
                                nc.tensor.matmul(
                                    pscs[h][:, off:],
                                    lhsT=v_res[:, b, h, t, :],
                                    rhs=emt[h][:, t, off:],
                                    start=(t == 0),
                                    stop=(t == nk - 1),
                                )

                        for t in range(nk):
                            off = off_of(t)
                            for h in range(HPC):
                                pss = psS.tile([128, 512], f32, tag="s")
                                nc.tensor.matmul(
                                    pss[:, off:],
                                    lhsT=kt_res[:, b, h, t * 128 : (t + 1) * 128],
                                    rhs=qt_res[
                                        :, b, h, qb * 512 + off : (qb + 1) * 512
                                    ],
                                    start=True,
                                    stop=True,
                                )
                                nc.scalar.activation(
                                    emt[h][:, t, off:], pss[:, off:], Exp, scale=SCALE
                                )
                                if t >= 4 * qb:
                                    # causal triangle on this diagonal chunk's
                                    # first 128 computed columns
                                    i = t - 4 * qb
                                    nc.gpsimd.tensor_mul(
                                        emt[h][:, t, off : off + 128],
                                        emt[h][:, t, off : off + 128],
                                        trix[:, i, off : off + 128],
                                    )
                            if t > 0:
                                issue_pv(t - 1)
                            # drain one pending output-projection tile per
                            # iteration from t=3 on (by then the previous
                            # block's normalize chain is done); the remainder
                            # drains at the block boundary, covering the PE
                            # while DVE runs this block's denominator fold
                            if t >= 3:
                                drain_phC(1)
                        issue_pv(nk - 1)

                        # denominator fold: halve E chunkwise down to one
                        # [128, 512] accumulator per head (wide DVE fp16 adds)
                        m = nk // 2
                        for h in range(HPC):
                            nc.vector.tensor_add(
                                acc[:, h, :m, :],
                                emt[h][:, :m, :],
                                emt[h][:, m : 2 * m, :],
                            )
                        while m > 1:
                            half = m // 2
                            nc.vector.tensor_add(
                                acc[:, :, :half, :],
                                acc[:, :, :half, :],
                                acc[:, :, half : 2 * half, :],
                            )
                            if m % 2 == 1:
                                nc.vector.tensor_add(
                                    acc[:, :, 0:1, :],
                                    acc[:, :, 0:1, :],
                                    acc[:, :, m - 1 : m, :],
                                )
                            m = half

                        # drain the remaining output-projection tiles BEFORE
                        # the denominator matmuls: their operands are ready,
                        # so the in-order PE queue chews them while DVE runs
                        # the fold chain (issuing the denominator matmul first
                        # would head-of-line block the PE on the fold)
                        drain_phC(len(pending))

                        for h in range(HPC):
                            psd = psD.tile(
                                [128, 512], f32, tag="d", name=f"d{b}_{qb}_{h}"
                            )
                            nc.tensor.matmul(
                                psd,
                                lhsT=ones_cm,
                                rhs=acc[:, h, 0, :],
                                start=True,
                                stop=True,
                            )
                            rec = recp.tile([128, 512], f32, tag="rec")
                            nc.vector.reciprocal_approx_fast(out=rec, in_=psd)
                            nc.vector.tensor_mul(
                                ctxs[h][:, qb * 512 : (qb + 1) * 512], pscs[h], rec
                            )
                        push_phC(b, qb, ctxs)
                # drain the final output-projection tiles
                drain_phC(len(pending))

    nc.finalize()
    return nc


def _get_nc(with_bias=False):
    if with_bias not in _built:
        _built[with_bias] = _build(with_bias)
    return _built[with_bias]


def kernel(hidden_states, attention_mask, Wq, bq, Wk, bk, Wv, bv, Wo, bo):
    hidden_states = np.asarray(hidden_states, dtype=np.float32)
    Wq, Wk, Wv, Wo = (np.asarray(w, dtype=np.float32) for w in (Wq, Wk, Wv, Wo))
    bq, bk, bv, bo = (np.asarray(v, dtype=np.float32) for v in (bq, bk, bv, bo))

    with_bias = bool(np.any(bq) or np.any(bk) or np.any(bv))

    x = hidden_states.reshape(T, D)
    # [KO, 128, T]: XT[ko, p, t] = x[t, 128*ko + p]
    xt = np.ascontiguousarray(x.T).reshape(KO, 128, T).astype(np.float16)

    # extended causal masks for the 4 diagonal-chunk offsets:
    # trix[p, i, f] = (p + 128*i <= f)
    p_idx = np.arange(128)[:, None, None]
    i_idx = np.arange(4)[None, :, None]
    f_idx = np.arange(512)[None, None, :]
    trix = (p_idx + 128 * i_idx <= f_idx).astype(np.float16)

    in_maps = []
    for c in range(NCORES):
        rows = slice(c * HPC * HD, (c + 1) * HPC * HD)
        wqt = np.ascontiguousarray(Wq[rows, :].T).reshape(KO, 128, HPC * HD)
        wkt = np.ascontiguousarray(Wk[rows, :].T).reshape(KO, 128, HPC * HD)
        wvt = np.ascontiguousarray(Wv[rows, :].T).reshape(KO, 128, HPC * HD)
        # WOT[p, h, n] = Wo[n, c*256 + h*128 + p]
        wot = np.ascontiguousarray(
            Wo[:, rows].T.reshape(HPC, 128, D).transpose(1, 0, 2)
        )
        bias = np.stack([bq[rows], bk[rows], bv[rows]])[None]
        in_maps.append(
            {
                "XT": xt,
                "WQT": wqt.astype(np.float16),
                "WKT": wkt.astype(np.float16),
                "WVT": wvt.astype(np.float16),
                "WOT": wot.astype(np.float16),
                "BIAS": bias.astype(np.float16),
                "TRIX": trix,
                "ONES": np.ones((128, 512), dtype=np.float16),
            }
        )

    res = run_bass_kernel_spmd(_get_nc(with_bias), in_maps, list(range(NCORES)))
    out = res.results[0]["OUT"].astype(np.float32)
    for c in range(1, NCORES):
        out += res.results[c]["OUT"].astype(np.float32)
    out += bo
    return out
